# revision 28
# baseline (speedup 1.0000x reference)
"""AttentionBasedAdapter Trainium2 kernel.

Data-parallel over the batch dim: 8 NeuronCores, 2 batches (2048 tokens)
per core. The small context-side projections are folded on host (exact
linear algebra, fp32) into two replicated tensors:

    MT  = (Wq @ (ce@Wk + bk)^T) / sqrt(A)     [D, C]
    bc  = ((ce@Wk + bk) @ bq) / sqrt(A)       [C]
    vWo = (ce@Wv + bv) @ Wo                   [C, P]

so the device per core computes (tokens on the free dim throughout,
no on-device transposes):

    sT   = MT_chunks^T @ xT  (+ bc as per-partition bias)   [c, tok]
    eT   = exp(sT)                     chunks [c, tok]
    den  = 1^T @ (sum of eT chunks)                          [1, tok]
    outT = (vWo^T-chunks @ eT) * (1/den bcast) + bo          [P, tok]

The softmax 1/den scaling is a per-token row scaling, so it commutes
past the (folded) Wo projection; attn rows summing to 1 makes the
folded bv/bk handling exact. Matmuls run in bf16 with fp32 PSUM
accumulation. Host pre-transposes/casts inputs, re-transposes output.
"""

from contextlib import ExitStack

import ml_dtypes
import numpy as np

import concourse.tile as tile
from concourse import bacc, bass_isa, bass_utils, mybir

BF16 = ml_dtypes.bfloat16

B, T, D = 16, 1024, 512
C, DC, A, P = 4096, 512, 512, 512
NCORE = 8
BPC = B // NCORE            # batches per core
TOK = BPC * T               # 2048 tokens per core
GRP = 1024                  # tokens processed per group
NG = TOK // GRP             # 4 groups
KD = D // 128               # 4 contraction chunks of 128
NCC = C // 128              # 32 context chunks of 128
SCALE = float(1.0 / np.sqrt(A))

F32 = mybir.dt.float32
BF = mybir.dt.bfloat16


def _build():
    nc = bacc.Bacc(
        "TRN2",
        target_bir_lowering=False,
        debug=False,
        enable_asserts=False,
        num_devices=NCORE,
    )
    xT = nc.dram_tensor("xT", [D, TOK], BF, kind="ExternalInput").ap()
    mTd = nc.dram_tensor("mT", [D, C], BF, kind="ExternalInput").ap()
    vwd = nc.dram_tensor("vw", [C, P], BF, kind="ExternalInput").ap()
    bcd = nc.dram_tensor("bc", [C], F32, kind="ExternalInput").ap()
    bod = nc.dram_tensor("bo", [P], F32, kind="ExternalInput").ap()
    outT = nc.dram_tensor("outT", [P, TOK], F32, kind="ExternalOutput").ap()

    with tile.TileContext(nc) as tc:
        with ExitStack() as ctx:
            consts = ctx.enter_context(tc.tile_pool(name="consts", bufs=1))
            big = ctx.enter_context(tc.tile_pool(name="big", bufs=1))
            sc = ctx.enter_context(tc.tile_pool(name="sc", bufs=1))
            xq = ctx.enter_context(tc.tile_pool(name="xq", bufs=2))
            ds_pool = ctx.enter_context(tc.tile_pool(name="ds", bufs=2))
            zs_pool = ctx.enter_context(tc.tile_pool(name="zs", bufs=2))
            rd_pool = ctx.enter_context(tc.tile_pool(name="rd", bufs=2))
            ps = ctx.enter_context(tc.tile_pool(name="ps", bufs=4, space="PSUM"))

            # ---- small constants first (sync HWDGE ring is FIFO) ----
            xT_r = xT.rearrange("(k p) t -> p k t", p=128)
            xT0_sb = xq.tile([128, KD, GRP], BF, tag="xT")
            nc.sync.dma_start(xT0_sb[:], xT_r[:, :, 0:GRP])
            bc_sb = consts.tile([128, NCC], F32, tag="bc")
            nc.sync.dma_start(bc_sb[:], bcd.rearrange("(i p) -> p i", p=128))
            bo_sb = consts.tile([128, KD], F32, tag="bo")
            nc.sync.dma_start(bo_sb[:], bod.rearrange("(m p) -> p m", p=128))

            # mT split by c-eighths: stage2's first chunks unblock after 0.5 MB
            # instead of 4 MB (Tile tracks per-region deps within the tile).
            mT_sb = big.tile([128, KD, C], BF, tag="mT")
            mT_r = mTd.rearrange("(m p) c -> p m c", p=128)
            CQ = C // 8
            for q in range(8):
                nc.sync.dma_start(
                    mT_sb[:, :, CQ * q : CQ * (q + 1)],
                    mT_r[:, :, CQ * q : CQ * (q + 1)],
                )
            vw_sb = big.tile([128, NCC, 512], BF, tag="vw")
            nc.sync.dma_start(vw_sb[:], vwd.rearrange("(i p) a -> p i a", p=128))

            for g in range(NG):
                # ---- load x^T for this token group ----
                if g == 0:
                    xT_sb = xT0_sb
                else:
                    xT_sb = xq.tile([128, KD, GRP], BF, tag="xT")
                    nc.sync.dma_start(xT_sb[:], xT_r[:, :, GRP * g : GRP * (g + 1)])

                # ---- scores^T chunks + exp; dsum accumulates exp chunks ----
                expT_sb = sc.tile([128, NCC, GRP], BF, tag="sc")
                dsum = ds_pool.tile([128, GRP], F32, tag="dsum")
                for i in range(NCC):
                    pscr = ps.tile([128, GRP], F32, tag="mm")
                    for m in range(KD):
                        # two N=512 matmuls share one LDWEIGHTS (same lhsT)
                        for h in range(2):
                            nc.tensor.matmul(
                                pscr[:, 512 * h : 512 * (h + 1)],
                                mT_sb[:, m, 128 * i : 128 * (i + 1)],
                                xT_sb[:, m, 512 * h : 512 * (h + 1)],
                                start=(m == 0),
                                stop=(m == KD - 1),
                            )
                    nc.scalar.activation(
                        expT_sb[:, i, :],
                        pscr[:],
                        mybir.ActivationFunctionType.Exp,
                        bias=bc_sb[:, i : i + 1],
                    )
                    if i == 1:
                        nc.vector.tensor_add(
                            dsum[:], expT_sb[:, 0, :], expT_sb[:, 1, :]
                        )
                    elif i > 1:
                        nc.vector.tensor_add(dsum[:], dsum[:], expT_sb[:, i, :])

                # ---- den = allreduce(dsum) over partitions; rden = 1/den ----
                den_bc = rd_pool.tile([128, GRP], F32, tag="denb")
                nc.gpsimd.partition_all_reduce(
                    den_bc[:], dsum[:], channels=128, reduce_op=bass_isa.ReduceOp.add
                )
                rden_bc = rd_pool.tile([128, GRP], F32, tag="rdenb")
                nc.vector.reciprocal(rden_bc[:], den_bc[:])

                # ---- outT chunks = (vWo^T @ eT) * rden + bo ----
                for m in range(KD):
                    po = ps.tile([128, GRP], F32, tag="mm")
                    for i in range(NCC):
                        for h in range(2):
                            nc.tensor.matmul(
                                po[:, 512 * h : 512 * (h + 1)],
                                vw_sb[:, i, 128 * m : 128 * (m + 1)],
                                expT_sb[:, i, 512 * h : 512 * (h + 1)],
                                start=(i == 0),
                                stop=(i == NCC - 1),
                            )
                    zt = zs_pool.tile([128, GRP], F32, tag="ztmp")
                    nc.vector.tensor_mul(zt[:], po[:], rden_bc[:])
                    zs = zs_pool.tile([128, GRP], F32, tag="zs")
                    nc.scalar.activation(
                        zs[:],
                        zt[:],
                        mybir.ActivationFunctionType.Identity,
                        bias=bo_sb[:, m : m + 1],
                    )
                    nc.sync.dma_start(
                        outT[128 * m : 128 * (m + 1), GRP * g : GRP * (g + 1)], zs[:]
                    )

    nc.compile()
    return nc


_CACHE = {}


def _get_nc():
    if "nc" not in _CACHE:
        _CACHE["nc"] = _build()
    return _CACHE["nc"]


def _prepare_in_maps(inputs):
    return _make_in_maps(**inputs)


def _make_in_maps(model_embed, context_embed, Wq, bq, Wk, bk, Wv, bv, Wo, bo):
    model_embed = np.asarray(model_embed, dtype=np.float32)
    context_embed = np.asarray(context_embed, dtype=np.float32)
    Wq, bq = np.asarray(Wq, np.float32), np.asarray(bq, np.float32)
    Wk, bk = np.asarray(Wk, np.float32), np.asarray(bk, np.float32)
    Wv, bv = np.asarray(Wv, np.float32), np.asarray(bv, np.float32)
    Wo, bo = np.asarray(Wo, np.float32), np.asarray(bo, np.float32)

    k_h = context_embed @ Wk + bk            # [C, A] fp32
    mT = (Wq @ k_h.T) * SCALE                # [D, C] fp32
    bc = (k_h @ bq) * SCALE                  # [C]    fp32
    vWo = (context_embed @ Wv + bv) @ Wo     # [C, P] fp32
    shared = {
        "mT": np.ascontiguousarray(mT).astype(BF16),
        "vw": vWo.astype(BF16),
        "bc": bc.astype(np.float32),
        "bo": bo.astype(np.float32),
    }
    in_maps = []
    for c in range(NCORE):
        xs = model_embed[BPC * c : BPC * (c + 1)].reshape(TOK, D)
        m = dict(shared)
        m["xT"] = np.ascontiguousarray(xs.T).astype(BF16)
        in_maps.append(m)
    return in_maps


def kernel(**inputs):
    nc = _get_nc()
    in_maps = _make_in_maps(**inputs)
    res = bass_utils.run_bass_kernel_spmd(nc, in_maps, core_ids=list(range(NCORE)))

    out = np.empty((B, T, P), dtype=np.float32)
    for c in range(NCORE):
        outT_c = res.results[c]["outT"]  # [P, TOK]
        out[BPC * c : BPC * (c + 1)] = outT_c.T.reshape(BPC, T, P)
    return out


# revision 29
# speedup vs baseline: 1.0102x; 1.0102x over previous
"""AttentionBasedAdapter Trainium2 kernel.

Data-parallel over the batch dim: 8 NeuronCores, 2 batches (2048 tokens)
per core. The small context-side projections are folded on host (exact
linear algebra, fp32) into two replicated tensors:

    MT  = (Wq @ (ce@Wk + bk)^T) / sqrt(A)     [D, C]
    bc  = ((ce@Wk + bk) @ bq) / sqrt(A)       [C]
    vWo = (ce@Wv + bv) @ Wo                   [C, P]

so the device per core computes (tokens on the free dim throughout,
no on-device transposes):

    sT   = MT_chunks^T @ xT  (+ bc as per-partition bias)   [c, tok]
    eT   = exp(sT)                     chunks [c, tok]
    den  = 1^T @ (sum of eT chunks)                          [1, tok]
    outT = (vWo^T-chunks @ eT) * (1/den bcast) + bo          [P, tok]

The softmax 1/den scaling is a per-token row scaling, so it commutes
past the (folded) Wo projection; attn rows summing to 1 makes the
folded bv/bk handling exact. Matmuls run in bf16 with fp32 PSUM
accumulation. Host pre-transposes/casts inputs, re-transposes output.
"""

from contextlib import ExitStack

import ml_dtypes
import numpy as np

import concourse.tile as tile
from concourse import bacc, bass_isa, bass_utils, mybir

BF16 = ml_dtypes.bfloat16

B, T, D = 16, 1024, 512
C, DC, A, P = 4096, 512, 512, 512
NCORE = 8
BPC = B // NCORE            # batches per core
TOK = BPC * T               # 2048 tokens per core
GRP = 512                   # tokens processed per group
NG = TOK // GRP             # 4 groups
KD = D // 128               # 4 contraction chunks of 128
NCC = C // 128              # 32 context chunks of 128
SCALE = float(1.0 / np.sqrt(A))

F32 = mybir.dt.float32
BF = mybir.dt.bfloat16


def _build():
    nc = bacc.Bacc(
        "TRN2",
        target_bir_lowering=False,
        debug=False,
        enable_asserts=False,
        num_devices=NCORE,
    )
    xT = nc.dram_tensor("xT", [D, TOK], BF, kind="ExternalInput").ap()
    mTd = nc.dram_tensor("mT", [D, C], BF, kind="ExternalInput").ap()
    vwd = nc.dram_tensor("vw", [C, P], BF, kind="ExternalInput").ap()
    bcd = nc.dram_tensor("bc", [C], F32, kind="ExternalInput").ap()
    bod = nc.dram_tensor("bo", [P], F32, kind="ExternalInput").ap()
    outT = nc.dram_tensor("outT", [P, TOK], F32, kind="ExternalOutput").ap()

    with tile.TileContext(nc) as tc:
        with ExitStack() as ctx:
            consts = ctx.enter_context(tc.tile_pool(name="consts", bufs=1))
            big = ctx.enter_context(tc.tile_pool(name="big", bufs=1))
            sc = ctx.enter_context(tc.tile_pool(name="sc", bufs=2))
            xq = ctx.enter_context(tc.tile_pool(name="xq", bufs=2))
            ds_pool = ctx.enter_context(tc.tile_pool(name="ds", bufs=2))
            zs_pool = ctx.enter_context(tc.tile_pool(name="zs", bufs=2))
            rd_pool = ctx.enter_context(tc.tile_pool(name="rd", bufs=2))
            ps = ctx.enter_context(tc.tile_pool(name="ps", bufs=8, space="PSUM"))

            # ---- small constants first (sync HWDGE ring is FIFO) ----
            xT_r = xT.rearrange("(k p) t -> p k t", p=128)
            xT0_sb = xq.tile([128, KD, GRP], BF, tag="xT")
            nc.sync.dma_start(xT0_sb[:], xT_r[:, :, 0:GRP])
            bc_sb = consts.tile([128, NCC], F32, tag="bc")
            nc.sync.dma_start(bc_sb[:], bcd.rearrange("(i p) -> p i", p=128))
            bo_sb = consts.tile([128, KD], F32, tag="bo")
            nc.sync.dma_start(bo_sb[:], bod.rearrange("(m p) -> p m", p=128))

            # mT split by c-eighths: stage2's first chunks unblock after 0.5 MB
            # instead of 4 MB (Tile tracks per-region deps within the tile).
            mT_sb = big.tile([128, KD, C], BF, tag="mT")
            mT_r = mTd.rearrange("(m p) c -> p m c", p=128)
            CQ = C // 8
            for q in range(8):
                nc.sync.dma_start(
                    mT_sb[:, :, CQ * q : CQ * (q + 1)],
                    mT_r[:, :, CQ * q : CQ * (q + 1)],
                )
            vw_sb = big.tile([128, NCC, 512], BF, tag="vw")
            nc.sync.dma_start(vw_sb[:], vwd.rearrange("(i p) a -> p i a", p=128))

            for g in range(NG):
                # ---- load x^T for this token group ----
                if g == 0:
                    xT_sb = xT0_sb
                else:
                    xT_sb = xq.tile([128, KD, GRP], BF, tag="xT")
                    nc.sync.dma_start(xT_sb[:], xT_r[:, :, GRP * g : GRP * (g + 1)])

                # ---- scores^T chunks + exp; dsum accumulates exp chunks ----
                expT_sb = sc.tile([128, NCC, GRP], BF, tag="sc")
                dsum = ds_pool.tile([128, GRP], F32, tag="dsum")
                for i in range(NCC):
                    pscr = ps.tile([128, GRP], F32, tag="mm")
                    for m in range(KD):
                        nc.tensor.matmul(
                            pscr[:],
                            mT_sb[:, m, 128 * i : 128 * (i + 1)],
                            xT_sb[:, m, :],
                            start=(m == 0),
                            stop=(m == KD - 1),
                        )
                    nc.scalar.activation(
                        expT_sb[:, i, :],
                        pscr[:],
                        mybir.ActivationFunctionType.Exp,
                        bias=bc_sb[:, i : i + 1],
                    )
                    if i == 1:
                        nc.vector.tensor_add(
                            dsum[:], expT_sb[:, 0, :], expT_sb[:, 1, :]
                        )
                    elif i > 1:
                        nc.vector.tensor_add(dsum[:], dsum[:], expT_sb[:, i, :])

                # ---- den = allreduce(dsum) over partitions; rden = 1/den ----
                den_bc = rd_pool.tile([128, GRP], F32, tag="denb")
                nc.gpsimd.partition_all_reduce(
                    den_bc[:], dsum[:], channels=128, reduce_op=bass_isa.ReduceOp.add
                )
                rden_bc = rd_pool.tile([128, GRP], F32, tag="rdenb")
                nc.vector.reciprocal(rden_bc[:], den_bc[:])

                # ---- outT chunks = (vWo^T @ eT) * rden + bo ----
                for m in range(KD):
                    po = ps.tile([128, GRP], F32, tag="mm")
                    for i in range(NCC):
                        nc.tensor.matmul(
                            po[:],
                            vw_sb[:, i, 128 * m : 128 * (m + 1)],
                            expT_sb[:, i, :],
                            start=(i == 0),
                            stop=(i == NCC - 1),
                        )
                    zt = zs_pool.tile([128, GRP], F32, tag="ztmp")
                    nc.vector.tensor_mul(zt[:], po[:], rden_bc[:])
                    zs = zs_pool.tile([128, GRP], F32, tag="zs")
                    nc.scalar.activation(
                        zs[:],
                        zt[:],
                        mybir.ActivationFunctionType.Identity,
                        bias=bo_sb[:, m : m + 1],
                    )
                    nc.sync.dma_start(
                        outT[128 * m : 128 * (m + 1), GRP * g : GRP * (g + 1)], zs[:]
                    )

    nc.compile()
    return nc


_CACHE = {}


def _get_nc():
    if "nc" not in _CACHE:
        _CACHE["nc"] = _build()
    return _CACHE["nc"]


def _prepare_in_maps(inputs):
    return _make_in_maps(**inputs)


def _make_in_maps(model_embed, context_embed, Wq, bq, Wk, bk, Wv, bv, Wo, bo):
    model_embed = np.asarray(model_embed, dtype=np.float32)
    context_embed = np.asarray(context_embed, dtype=np.float32)
    Wq, bq = np.asarray(Wq, np.float32), np.asarray(bq, np.float32)
    Wk, bk = np.asarray(Wk, np.float32), np.asarray(bk, np.float32)
    Wv, bv = np.asarray(Wv, np.float32), np.asarray(bv, np.float32)
    Wo, bo = np.asarray(Wo, np.float32), np.asarray(bo, np.float32)

    k_h = context_embed @ Wk + bk            # [C, A] fp32
    mT = (Wq @ k_h.T) * SCALE                # [D, C] fp32
    bc = (k_h @ bq) * SCALE                  # [C]    fp32
    vWo = (context_embed @ Wv + bv) @ Wo     # [C, P] fp32
    shared = {
        "mT": np.ascontiguousarray(mT).astype(BF16),
        "vw": vWo.astype(BF16),
        "bc": bc.astype(np.float32),
        "bo": bo.astype(np.float32),
    }
    in_maps = []
    for c in range(NCORE):
        xs = model_embed[BPC * c : BPC * (c + 1)].reshape(TOK, D)
        m = dict(shared)
        m["xT"] = np.ascontiguousarray(xs.T).astype(BF16)
        in_maps.append(m)
    return in_maps


def kernel(**inputs):
    nc = _get_nc()
    in_maps = _make_in_maps(**inputs)
    res = bass_utils.run_bass_kernel_spmd(nc, in_maps, core_ids=list(range(NCORE)))

    out = np.empty((B, T, P), dtype=np.float32)
    for c in range(NCORE):
        outT_c = res.results[c]["outT"]  # [P, TOK]
        out[BPC * c : BPC * (c + 1)] = outT_c.T.reshape(BPC, T, P)
    return out


# revision 33
# speedup vs baseline: 1.2014x; 1.1893x over previous
"""AttentionBasedAdapter Trainium2 kernel.

Data-parallel over the batch dim: 8 NeuronCores, 2 batches (2048 tokens)
per core. The small context-side projections are folded on host (exact
linear algebra, fp32) into two replicated tensors:

    MT  = (Wq @ (ce@Wk + bk)^T) / sqrt(A)     [D, C]
    bc  = ((ce@Wk + bk) @ bq) / sqrt(A)       [C]
    vWo = (ce@Wv + bv) @ Wo                   [C, P]

so the device per core computes (tokens on the free dim throughout,
no on-device transposes):

    sT   = MT_chunks^T @ xT  (+ bc as per-partition bias)   [c, tok]
    eT   = exp(sT)                     chunks [c, tok]
    den  = 1^T @ (sum of eT chunks)                          [1, tok]
    outT = (vWo^T-chunks @ eT) * (1/den bcast) + bo          [P, tok]

The softmax 1/den scaling is a per-token row scaling, so it commutes
past the (folded) Wo projection; attn rows summing to 1 makes the
folded bv/bk handling exact. Matmuls run in bf16 with fp32 PSUM
accumulation. Host pre-transposes/casts inputs, re-transposes output.
"""

from contextlib import ExitStack

import ml_dtypes
import numpy as np

import concourse.tile as tile
from concourse import bacc, bass_isa, bass_utils, mybir

BF16 = ml_dtypes.bfloat16

B, T, D = 16, 1024, 512
C, DC, A, P = 4096, 512, 512, 512
NCORE = 8
BPC = B // NCORE            # batches per core
TOK = BPC * T               # 2048 tokens per core
GRP = 512                   # tokens processed per group
NG = TOK // GRP             # 4 groups
KD = D // 128               # 4 contraction chunks of 128
NCC = C // 128              # 32 context chunks of 128
SCALE = float(1.0 / np.sqrt(A))

F32 = mybir.dt.float32
BF = mybir.dt.bfloat16


def _build():
    nc = bacc.Bacc(
        "TRN2",
        target_bir_lowering=False,
        debug=False,
        enable_asserts=False,
        num_devices=NCORE,
    )
    # inputs come pre-arranged from host so every DMA reads long contiguous
    # runs per partition (1 KB packets on the FIFO HWDGE ring were the
    # startup bottleneck)
    xT = nc.dram_tensor("xT", [128, NG, KD, GRP], BF, kind="ExternalInput").ap()
    mTd = nc.dram_tensor("mT", [128, 8, KD, C // 8], BF, kind="ExternalInput").ap()
    vwd = nc.dram_tensor("vw", [128, NCC, 512], BF, kind="ExternalInput").ap()
    bcd = nc.dram_tensor("bc", [C], F32, kind="ExternalInput").ap()
    bod = nc.dram_tensor("bo", [P], F32, kind="ExternalInput").ap()
    outT = nc.dram_tensor("outT", [P, TOK], F32, kind="ExternalOutput").ap()

    with tile.TileContext(nc) as tc:
        with ExitStack() as ctx:
            consts = ctx.enter_context(tc.tile_pool(name="consts", bufs=1))
            big = ctx.enter_context(tc.tile_pool(name="big", bufs=1))
            sc = ctx.enter_context(tc.tile_pool(name="sc", bufs=2))
            xq = ctx.enter_context(tc.tile_pool(name="xq", bufs=2))
            ds_pool = ctx.enter_context(tc.tile_pool(name="ds", bufs=2))
            zs_pool = ctx.enter_context(tc.tile_pool(name="zs", bufs=2))
            rd_pool = ctx.enter_context(tc.tile_pool(name="rd", bufs=2))
            ps = ctx.enter_context(tc.tile_pool(name="ps", bufs=8, space="PSUM"))

            # ---- small constants first (sync HWDGE ring is FIFO) ----
            xT0_sb = xq.tile([128, KD, GRP], BF, tag="xT")
            nc.sync.dma_start(xT0_sb[:], xT[:, 0, :, :])
            bc_sb = consts.tile([128, NCC], F32, tag="bc")
            nc.sync.dma_start(bc_sb[:], bcd.rearrange("(i p) -> p i", p=128))
            bo_sb = consts.tile([128, KD], F32, tag="bo")
            nc.sync.dma_start(bo_sb[:], bod.rearrange("(m p) -> p m", p=128))

            # mT split by c-eighths: stage2's first chunks unblock after 0.5 MB
            # instead of 4 MB (Tile tracks per-region deps within the tile).
            mT_sb = big.tile([128, KD, C], BF, tag="mT")
            CQ = C // 8
            for q in range(8):
                nc.sync.dma_start(
                    mT_sb[:, :, CQ * q : CQ * (q + 1)], mTd[:, q, :, :]
                )
            vw_sb = big.tile([128, NCC, 512], BF, tag="vw")
            nc.sync.dma_start(vw_sb[:], vwd[:])

            for g in range(NG):
                # ---- load x^T for this token group ----
                if g == 0:
                    xT_sb = xT0_sb
                else:
                    xT_sb = xq.tile([128, KD, GRP], BF, tag="xT")
                    nc.sync.dma_start(xT_sb[:], xT[:, g, :, :])

                # ---- scores^T chunks + exp; dsum accumulates exp chunks ----
                expT_sb = sc.tile([128, NCC, GRP], BF, tag="sc")
                dsum = ds_pool.tile([128, GRP], F32, tag="dsum")
                for i in range(NCC):
                    pscr = ps.tile([128, GRP], F32, tag="mm")
                    for m in range(KD):
                        nc.tensor.matmul(
                            pscr[:],
                            mT_sb[:, m, 128 * i : 128 * (i + 1)],
                            xT_sb[:, m, :],
                            start=(m == 0),
                            stop=(m == KD - 1),
                        )
                    nc.scalar.activation(
                        expT_sb[:, i, :],
                        pscr[:],
                        mybir.ActivationFunctionType.Exp,
                        bias=bc_sb[:, i : i + 1],
                    )
                    if i == 1:
                        nc.vector.tensor_add(
                            dsum[:], expT_sb[:, 0, :], expT_sb[:, 1, :]
                        )
                    elif i > 1:
                        nc.vector.tensor_add(dsum[:], dsum[:], expT_sb[:, i, :])

                # ---- den = allreduce(dsum) over partitions; rden = 1/den ----
                den_bc = rd_pool.tile([128, GRP], F32, tag="denb")
                nc.gpsimd.partition_all_reduce(
                    den_bc[:], dsum[:], channels=128, reduce_op=bass_isa.ReduceOp.add
                )
                rden_bc = rd_pool.tile([128, GRP], F32, tag="rdenb")
                nc.vector.reciprocal(rden_bc[:], den_bc[:])

                # ---- outT chunks = (vWo^T @ eT) * rden + bo ----
                for m in range(KD):
                    po = ps.tile([128, GRP], F32, tag="mm")
                    for i in range(NCC):
                        nc.tensor.matmul(
                            po[:],
                            vw_sb[:, i, 128 * m : 128 * (m + 1)],
                            expT_sb[:, i, :],
                            start=(i == 0),
                            stop=(i == NCC - 1),
                        )
                    zt = zs_pool.tile([128, GRP], F32, tag="ztmp")
                    nc.vector.tensor_mul(zt[:], po[:], rden_bc[:])
                    zs = zs_pool.tile([128, GRP], F32, tag="zs")
                    nc.scalar.activation(
                        zs[:],
                        zt[:],
                        mybir.ActivationFunctionType.Identity,
                        bias=bo_sb[:, m : m + 1],
                    )
                    nc.sync.dma_start(
                        outT[128 * m : 128 * (m + 1), GRP * g : GRP * (g + 1)], zs[:]
                    )

    nc.compile()
    return nc


_CACHE = {}


def _get_nc():
    if "nc" not in _CACHE:
        _CACHE["nc"] = _build()
    return _CACHE["nc"]


def _prepare_in_maps(inputs):
    return _make_in_maps(**inputs)


def _make_in_maps(model_embed, context_embed, Wq, bq, Wk, bk, Wv, bv, Wo, bo):
    model_embed = np.asarray(model_embed, dtype=np.float32)
    context_embed = np.asarray(context_embed, dtype=np.float32)
    Wq, bq = np.asarray(Wq, np.float32), np.asarray(bq, np.float32)
    Wk, bk = np.asarray(Wk, np.float32), np.asarray(bk, np.float32)
    Wv, bv = np.asarray(Wv, np.float32), np.asarray(bv, np.float32)
    Wo, bo = np.asarray(Wo, np.float32), np.asarray(bo, np.float32)

    k_h = context_embed @ Wk + bk            # [C, A] fp32
    mT = (Wq @ k_h.T) * SCALE                # [D, C] fp32
    bc = (k_h @ bq) * SCALE                  # [C]    fp32
    vWo = (context_embed @ Wv + bv) @ Wo     # [C, P] fp32

    # pre-arrange so device DMAs read long contiguous runs per partition:
    #   mT  [D=KD*128, C]   -> [128, 8, KD, C/8]
    #   vWo [C=NCC*128, P]  -> [128, NCC, P]
    #   x^T [D=KD*128, TOK] -> [128, NG, KD, GRP]
    mT_a = np.ascontiguousarray(
        mT.astype(BF16).reshape(KD, 128, 8, C // 8).transpose(1, 2, 0, 3)
    )
    vw_a = np.ascontiguousarray(
        vWo.astype(BF16).reshape(NCC, 128, P).transpose(1, 0, 2)
    )
    shared = {
        "mT": mT_a,
        "vw": vw_a,
        "bc": bc.astype(np.float32),
        "bo": bo.astype(np.float32),
    }
    in_maps = []
    for c in range(NCORE):
        xs = model_embed[BPC * c : BPC * (c + 1)].reshape(TOK, D)
        xT_a = np.ascontiguousarray(
            xs.T.astype(BF16).reshape(KD, 128, NG, GRP).transpose(1, 2, 0, 3)
        )
        m = dict(shared)
        m["xT"] = xT_a
        in_maps.append(m)
    return in_maps


def kernel(**inputs):
    nc = _get_nc()
    in_maps = _make_in_maps(**inputs)
    res = bass_utils.run_bass_kernel_spmd(nc, in_maps, core_ids=list(range(NCORE)))

    out = np.empty((B, T, P), dtype=np.float32)
    for c in range(NCORE):
        outT_c = res.results[c]["outT"]  # [P, TOK]
        out[BPC * c : BPC * (c + 1)] = outT_c.T.reshape(BPC, T, P)
    return out
